# revision 1
# baseline (speedup 1.0000x reference)
"""Dispersion loss kernel for 8x TRN2 NeuronCores (Bass/Tile).

Math: rows of class_centroid [8192, 2048] are L2-normalized; the loss is
  mean_i( sum_j exp(-||xn_i - xn_j||^2) / (N-1) )
    = (1/(N*(N-1))) * sum_{i,j} exp(2*cos_ij - 2)       (cos_ij = xn_i . xn_j)

Since only the total sum is needed, we exploit symmetry: with 16 row-blocks
of 512, only block pairs (b, b+d mod 16) for d=0..8 are computed (d=8 pairs
are covered twice and down-weighted on the host). Each core c loads the 10
consecutive blocks 2c..2c+9 (mod 16) and runs the SAME program (SPMD) over a
fixed list of 18 slot pairs; per-core slot->block mapping makes the union
cover all 136 unordered block pairs.

Per core: load raw rows with SWDGE fp32->bf16 cast; row sum-of-squares via
ScalarE Square with fused accumulate; rinv' = 16*rsqrt(ssq) computed as
exp(-0.5*ln(ssq)+ln 16) so the whole kernel stays on one ACT table set (the
16x scale feeds fp8 quantization); DVE row-scale; DMA-xbar transpose into
feature-major [128, D/128, 512] tiles; DVE cast to fp8-e4m3 (SBUF-resident,
~80KB/partition for 10 blocks). Gram tiles run as fp8 DoubleRow matmuls
(K=256 per instruction, 8 accumulating matmuls per [128,512] PSUM tile),
with pairs processed in sj-trios sharing one stationary load; a post-schedule
pass (_dedup_ldweights) strips the redundant LDWEIGHTS. Epilogue
exp(2G/256 - 2) on ScalarE with fused row-sum accumulate (explicit min(.,1)
clamp on diagonal tiles only). Host reduces the 8 partial tensors in float64.

The walrus build in this container predates this bass: _sem_clear_compat and
_split_multi_waits patch around unsupported opcodes (see memory:
trn2-walrus-compat).
"""

import numpy as np

import concourse.bass as bass
import concourse.mybir as mybir
from concourse.tile import TileContext
from concourse.bass_utils import run_bass_kernel_spmd

F32 = mybir.dt.float32
BF16 = mybir.dt.bfloat16
FP8 = mybir.dt.float8e4
FP8_SCALE = 16.0


# --------------------------------------------------------------------------
# Compatibility shims for the walrus compiler build in this container:
# 1) EVENT_SEMAPHORE_RANGE_CLEAR (opcode 176) is not understood -> emit
#    per-semaphore EventSemaphore sem-wr-imm 0 instead.
# 2) Instructions with >1 sync waits ("Too many sync wait commands") ->
#    split extra waits onto single-wait EventSemaphore carriers.
# --------------------------------------------------------------------------
def _sem_clear_compat(self, sem):
    nums = (
        list(sem) if isinstance(sem, range)
        else [sem.num if hasattr(sem, "num") else int(sem)]
    )
    inst = None
    for n in nums:
        inst = mybir.InstEventSemaphore(
            name=f"semclr_{self.bass.next_id()}",
            engine=self.engine,
            ins=[],
            outs=[],
            sync_info=mybir.SyncInfo(
                on_wait=[],
                on_update=[
                    mybir.SyncUpdate(
                        sync_type="semaphore",
                        id=n,
                        ant_name=f"semclr{n}",
                        update_mode="sem-wr-imm",
                        update_value=0,
                    )
                ],
            ),
            bass_nofuse=True,
        )
        self.add_instruction(inst)
    return inst


bass.BassGpSimd.sem_clear = _sem_clear_compat


def _dedup_ldweights(nc):
    """Remove consecutive PE LDWEIGHTS with identical source APs: the weights
    are already resident in the array, so repeated loads between matmuls that
    share a stationary tile are pure overhead. Non-empty sync_info on removed
    loads is preserved on a zero-cost EventSemaphore carrier."""
    def sig(i):
        ap = i.ins[0]
        return (
            getattr(ap, "memref", None), getattr(ap, "offset", None),
            str(getattr(ap, "ap", None)), str(getattr(ap, "dtype", None)),
            i.tile_position, i.perf_mode, i.is_transpose,
        )
    removed = 0
    for bb in nc.m.functions[0].blocks:
        new = []
        last = None
        for inst in bb.instructions:
            tn = type(inst).__name__
            if tn == "InstLdweights":
                s_ = sig(inst)
                if last is not None and s_ == last:
                    si_ = getattr(inst, "sync_info", None)
                    if si_ is not None and (si_.on_wait or si_.on_update):
                        new.append(mybir.InstEventSemaphore(
                            name=f"ldwdedup_{nc.next_id()}",
                            engine=inst.engine, ins=[], outs=[],
                            sync_info=si_, bass_nofuse=True,
                        ))
                    removed += 1
                    continue
                last = s_
            new.append(inst)
        bb.instructions[:] = new
    return removed


def _split_multi_waits(nc):
    for bb in nc.m.functions[0].blocks:
        new = []
        for inst in bb.instructions:
            si = getattr(inst, "sync_info", None)
            if si is not None and si.on_wait is not None and len(si.on_wait) > 1:
                waits = list(si.on_wait)
                for w in waits[:-1]:
                    carrier = mybir.InstEventSemaphore(
                        name=f"waitsplit_{nc.next_id()}",
                        engine=inst.engine,
                        ins=[],
                        outs=[],
                        sync_info=mybir.SyncInfo(on_wait=[w], on_update=[]),
                        bass_nofuse=True,
                    )
                    new.append(carrier)
                si.on_wait[:] = waits[-1:]
            new.append(inst)
        bb.instructions[:] = new

N_ROWS = 8192
D = 2048
NB = 16          # row blocks
RPB = 512        # rows per block
SLOTS = 10       # blocks cached per core
N_CORES = 8

# Fixed slot-pair list (si = stationary/m-rows, sj = moving/n-cols).
# Ordered so early pairs touch early slots (pipelines with block loads).
PAIRS = [(0, 0), (1, 1), (0, 1)]
for _k in range(2, 9):
    PAIRS += [(0, _k), (1, _k)]
PAIRS += [(1, 9)]
assert len(PAIRS) == 18


def slot_blocks(core):
    """Global block index for each slot on a given core."""
    return [(2 * core + k) % NB for k in range(SLOTS)]


def pair_weight(si, sj):
    """Host-side weight for one slot pair: diag=1, cross d<8 -> 2,
    d=8 cross pairs are computed twice globally -> 1 each."""
    if si == sj:
        return 1.0
    d = sj - si
    return 1.0 if d == 8 else 2.0


def build_program(rpb=RPB, d=D, slots=SLOTS, pairs=PAIRS, psum_bufs=6,
                  phase0=True, phase1=True, loop_n=None, use_fp8=True):
    """Uniform SPMD program. Input: xin [slots, rpb, d] f32 (per-core blocks).
    Output: partials [128, len(pairs)] f32: per-partition sums of exp(2G-2)
    over each block-pair tile."""
    rt = rpb // 128   # 128-row subtiles per block
    kc = d // 128     # contraction chunks
    nc = bass.Bass()
    xin = nc.declare_dram_parameter("xin", [slots, rpb, d], F32, isOutput=False)
    pout = nc.declare_dram_parameter(
        "partials", [128, rt * len(pairs)], F32, isOutput=True
    )

    mult = mybir.AluOpType.mult
    add = mybir.AluOpType.add
    amin = mybir.AluOpType.min
    Exp = mybir.ActivationFunctionType.Exp
    Ln = mybir.ActivationFunctionType.Ln
    Square = mybir.ActivationFunctionType.Square

    with TileContext(nc) as tc:
        with (
            tc.tile_pool(name="xnt", bufs=1) as xnt_pool,
            tc.tile_pool(name="stage", bufs=6 if use_fp8 else 4) as stage_pool,
            tc.tile_pool(name="xns", bufs=2) as xn_pool,
            tc.tile_pool(name="dump", bufs=2) as dump_pool,
            tc.tile_pool(name="ediag", bufs=1) as ediag_pool,
            tc.tile_pool(name="scr", bufs=1) as scr_pool,
            tc.tile_pool(name="small", bufs=4) as small_pool,
            tc.tile_pool(name="acc", bufs=1) as acc_pool,
            tc.tile_pool(name="gpsum", bufs=psum_bufs, space="PSUM") as gpsum_pool,
        ):
            partials = acc_pool.tile([128, rt * len(pairs)], F32, tag="partials")
            bias_t = acc_pool.tile([128, 1], F32, tag="biasneg2")
            nc.vector.memset(bias_t, -2.0)
            xnt_dt = FP8 if use_fp8 else BF16
            xnt = [
                xnt_pool.tile(
                    [128, kc, rpb], xnt_dt, tag=f"xnt{s}", name=f"xnt{s}"
                )
                for s in range(slots)
            ]
            lnS = acc_pool.tile([128, 1], F32, tag="lnS")
            nc.vector.memset(lnS, float(np.log(FP8_SCALE)))

            import contextlib
            loop_ctx = (
                tc.For_i(0, loop_n, 1) if loop_n else contextlib.nullcontext()
            )
            with loop_ctx:
                # ---- Phase 0: load + normalize + transpose each slot block ----
                for s in range(slots if phase0 else 0):
                    ssqb = small_pool.tile([128, rt], F32, tag="ssqb")
                    xbs = []
                    for r in range(rt):
                        xb = stage_pool.tile([128, d], BF16, tag="xb")
                        # SWDGE casts f32 DRAM -> bf16 SBUF during the DMA.
                        nc.gpsimd.dma_start(
                            out=xb, in_=xin[s, r * 128 : (r + 1) * 128, :]
                        )
                        sqs = dump_pool.tile([128, d], BF16, tag="dump")
                        nc.scalar.activation(
                            sqs, xb, Square, accum_out=ssqb[:, r : r + 1]
                        )
                        xbs.append(xb)
                    # rinv' = S*rsqrt(ssq) = exp(-0.5*ln(ssq) + ln S);
                    # Ln+Exp share one ACT table set
                    lssq = small_pool.tile([128, rt], F32, tag="lssq")
                    nc.scalar.activation(lssq, ssqb, Ln)
                    rinvb = small_pool.tile([128, rt], BF16, tag="rinvb")
                    if use_fp8:
                        nc.scalar.activation(rinvb, lssq, Exp, scale=-0.5,
                                             bias=lnS)
                    else:
                        nc.scalar.activation(rinvb, lssq, Exp, scale=-0.5)
                    if use_fp8:
                        xntb = xn_pool.tile([128, kc, rpb], BF16, tag="xntb")
                    for r in range(rt):
                        xn = xn_pool.tile([128, d], BF16, tag="xn")
                        nc.vector.tensor_tensor(
                            out=xn, in0=xbs[r],
                            in1=rinvb[:, r : r + 1].to_broadcast((128, d)), op=mult
                        )
                        tdst = xntb if use_fp8 else xnt[s]
                        # xbar transpose (2B only): out[p, c, rr] = xn[rr, 128c+p]
                        nc.sync.dma_start_transpose(
                            out=tdst[:, :, r * 128 : (r + 1) * 128], in_=xn
                        )
                        if use_fp8:
                            # cast transposed slice to fp8 for DoubleRow matmul
                            nc.vector.tensor_copy(
                                xnt[s][:, :, r * 128 : (r + 1) * 128],
                                xntb[:, :, r * 128 : (r + 1) * 128],
                            )

                # ---- Phase 1: sj-grouped so one stationary serves 3 matmuls ----
                groups = []
                for si_ in (0, 1):
                    sjs = [sj_ for (a_, sj_) in pairs if a_ == si_]
                    for gi in range(0, len(sjs), 3):
                        groups.append((si_, sjs[gi : gi + 3]))
                for si, grp in (groups if phase1 else []):
                    for mi in range(rt):
                        gs = []
                        for sj in grp:
                            gt = gpsum_pool.tile(
                                [128, rpb], F32, tag="g", name=f"g{si}_{sj}_{mi}"
                            )
                            gs.append(gt)
                        if use_fp8:
                            kc8 = kc // 2
                            for k in range(kc8):
                                for j, sj in enumerate(grp):
                                    nc.tensor.matmul(
                                        gs[j],
                                        xnt[si][:, 2 * k : 2 * k + 2,
                                                mi * 128 : (mi + 1) * 128],
                                        xnt[sj][:, 2 * k : 2 * k + 2, :],
                                        start=(k == 0),
                                        stop=(k == kc8 - 1),
                                        perf_mode=mybir.MatmulPerfMode.DoubleRow,
                                    )
                        else:
                            for k in range(kc):
                                for j, sj in enumerate(grp):
                                    nc.tensor.matmul(
                                        gs[j],
                                        xnt[si][:, k, mi * 128 : (mi + 1) * 128],
                                        xnt[sj][:, k, :],
                                        start=(k == 0),
                                        stop=(k == kc - 1),
                                    )
                        for j, sj in enumerate(grp):
                            t = pairs.index((si, sj))
                            pcol = t * rt + mi
                            if si == sj:
                                # diag needs the max(d2,0) clamp:
                                # e = min(exp(2G-2), 1)
                                e = ediag_pool.tile([128, rpb], F32, tag="ediag")
                                esc = (2.0 / (FP8_SCALE * FP8_SCALE)
                                       if use_fp8 else 2.0)
                                nc.scalar.activation(
                                    e, gs[j], Exp, bias=bias_t, scale=esc,
                                )
                                scr = scr_pool.tile([128, rpb], BF16, tag="scr")
                                nc.vector.tensor_tensor(
                                    out=scr, in0=e,
                                    in1=nc.const_aps.tensor(1.0, (128, rpb)),
                                    op=amin,
                                )
                                nc.vector.tensor_reduce(
                                    out=partials[:, pcol : pcol + 1], in_=scr,
                                    axis=mybir.AxisListType.X, op=add,
                                )
                            else:
                                edump = dump_pool.tile(
                                    [128, rpb], BF16, tag="dump"
                                )
                                esc = (2.0 / (FP8_SCALE * FP8_SCALE)
                                       if use_fp8 else 2.0)
                                nc.scalar.activation(
                                    edump, gs[j], Exp, bias=bias_t, scale=esc,
                                    accum_out=partials[:, pcol : pcol + 1],
                                )

            nc.sync.dma_start(out=pout[:, :], in_=partials)
    n_dedup = _dedup_ldweights(nc)
    _split_multi_waits(nc)
    return nc


_PROGRAM_CACHE = {}


def _get_program():
    if "nc" not in _PROGRAM_CACHE:
        _PROGRAM_CACHE["nc"] = build_program()
    return _PROGRAM_CACHE["nc"]


def shard_inputs(x):
    """x: [8192, 2048] f32 -> per-core input dicts."""
    blocks = x.reshape(NB, RPB, D)
    in_maps = []
    for c in range(N_CORES):
        sel = np.ascontiguousarray(blocks[slot_blocks(c)])
        in_maps.append({"xin": sel})
    return in_maps


def reduce_partials(results, rt=RPB // 128):
    """results: list of dicts with 'partials' [128, rt*18] f32 -> scalar."""
    w = np.array([pair_weight(si, sj) for (si, sj) in PAIRS], dtype=np.float64)
    total = 0.0
    for res in results:
        p = res["partials"].astype(np.float64).reshape(128, len(PAIRS), rt)
        total += (p.sum(axis=(0, 2)) * w).sum()
    return total / (N_ROWS * (N_ROWS - 1))


def kernel(class_centroid: np.ndarray) -> np.ndarray:
    x = np.asarray(class_centroid, dtype=np.float32)
    assert x.shape == (N_ROWS, D)
    nc = _get_program()
    in_maps = shard_inputs(x)
    out = run_bass_kernel_spmd(nc, in_maps, list(range(N_CORES)))
    total = reduce_partials(out.results)
    return np.float32(total)



# revision 17
# speedup vs baseline: 3.0319x; 3.0319x over previous
"""Dispersion loss kernel for 8x TRN2 NeuronCores (Bass/Tile).

Math: rows of class_centroid [8192, 2048] are L2-normalized; the loss is
  mean_i( sum_j exp(-||xn_i - xn_j||^2) / (N-1) )
    = (1/(N*(N-1))) * sum_{i,j} exp(2*cos_ij - 2)       (cos_ij = xn_i . xn_j)

Decomposition: 16 row-blocks of 512. Cores use CONSECUTIVE shifts (core c
covers blocks c + S[k] mod 16 for the 8-element base set
S = {0,1,2,4} u {8,9,10,12}; {0,1,2,4} is a perfect difference basis of Z8),
which yields an EXACT cover: 17 slot-pairs per core x 8 cores = 136 distinct
block pairs, every unordered cross pair computed exactly once (weight 2 on
the host), both diagonal loops once (weight 1). No d=8 double count.

Per core: 8 blocks are loaded raw with SWDGE fp32->bf16 cast; row
sum-of-squares in ONE fused DVE pass (tensor_tensor_reduce mult/add);
rinv' = 16*rsqrt(ssq) via exp(-0.5*ln+ln16) on ACT (one table set); the
normalize-scale is FUSED into the fp8 cast (one DVE tensor_tensor per
subtile); the fp8 data is transposed through the DMA xbar as 2-byte units
(half the transpose bytes of bf16) giving a feature-PAIR-major layout
[128, kc, 512 rows, 2]. DoubleRow matmuls consume it with a pair-interleaved
K access pattern (j stride 1 byte) - the (partition, j) -> feature bijection
is consistent on both operands, so the contraction is exact. Epilogue
exp(2G/256 - 2) on ACT with fused row-sum accumulate; diagonal tiles get a
fused min(e,1)+row-sum on DVE (tensor_tensor_reduce min/add). Host reduces
the 8 partial tensors in float64.

The walrus build in this container predates this bass: _sem_clear_compat and
_split_multi_waits patch around unsupported opcodes.
"""

import numpy as np

import concourse.bass as bass
import concourse.mybir as mybir
from concourse.tile import TileContext
from concourse.bass_utils import run_bass_kernel_spmd

F32 = mybir.dt.float32
BF16 = mybir.dt.bfloat16
FP8 = mybir.dt.float8e4
FP8_SCALE = 16.0


# --------------------------------------------------------------------------
# Compatibility shims for the walrus compiler build in this container:
# 1) EVENT_SEMAPHORE_RANGE_CLEAR (opcode 176) is not understood -> emit
#    per-semaphore EventSemaphore sem-wr-imm 0 instead.
# 2) Instructions with >1 sync waits ("Too many sync wait commands") ->
#    split extra waits onto single-wait EventSemaphore carriers.
# --------------------------------------------------------------------------
def _sem_clear_compat(self, sem):
    nums = (
        list(sem) if isinstance(sem, range)
        else [sem.num if hasattr(sem, "num") else int(sem)]
    )
    inst = None
    for n in nums:
        inst = mybir.InstEventSemaphore(
            name=f"semclr_{self.bass.next_id()}",
            engine=self.engine,
            ins=[],
            outs=[],
            sync_info=mybir.SyncInfo(
                on_wait=[],
                on_update=[
                    mybir.SyncUpdate(
                        sync_type="semaphore",
                        id=n,
                        ant_name=f"semclr{n}",
                        update_mode="sem-wr-imm",
                        update_value=0,
                    )
                ],
            ),
            bass_nofuse=True,
        )
        self.add_instruction(inst)
    return inst


bass.BassGpSimd.sem_clear = _sem_clear_compat


def _dedup_ldweights(nc):
    """Remove consecutive PE LDWEIGHTS with identical source APs: the weights
    are already resident in the array, so repeated loads between matmuls that
    share a stationary tile are pure overhead. Non-empty sync_info on removed
    loads is preserved on a zero-cost EventSemaphore carrier."""
    def sig(i):
        ap = i.ins[0]
        return (
            getattr(ap, "memref", None), getattr(ap, "offset", None),
            str(getattr(ap, "ap", None)), str(getattr(ap, "dtype", None)),
            i.tile_position, i.perf_mode, i.is_transpose,
        )
    removed = 0
    for bb in nc.m.functions[0].blocks:
        new = []
        last = None
        for inst in bb.instructions:
            tn = type(inst).__name__
            if tn == "InstLdweights":
                s_ = sig(inst)
                if last is not None and s_ == last:
                    si_ = getattr(inst, "sync_info", None)
                    if si_ is not None and (si_.on_wait or si_.on_update):
                        new.append(mybir.InstEventSemaphore(
                            name=f"ldwdedup_{nc.next_id()}",
                            engine=inst.engine, ins=[], outs=[],
                            sync_info=si_, bass_nofuse=True,
                        ))
                    removed += 1
                    continue
                last = s_
            new.append(inst)
        bb.instructions[:] = new
    return removed


def _split_multi_waits(nc):
    for bb in nc.m.functions[0].blocks:
        new = []
        for inst in bb.instructions:
            si = getattr(inst, "sync_info", None)
            if si is not None and si.on_wait is not None and len(si.on_wait) > 1:
                waits = list(si.on_wait)
                for w in waits[:-1]:
                    carrier = mybir.InstEventSemaphore(
                        name=f"waitsplit_{nc.next_id()}",
                        engine=inst.engine,
                        ins=[],
                        outs=[],
                        sync_info=mybir.SyncInfo(on_wait=[w], on_update=[]),
                        bass_nofuse=True,
                    )
                    new.append(carrier)
                si.on_wait[:] = waits[-1:]
            new.append(inst)
        bb.instructions[:] = new


N_ROWS = 8192
D = 2048
NB = 16          # row blocks
RPB = 512        # rows per block
RT = RPB // 128  # 128-row subtiles per block
KC = D // 256    # fp8 contraction chunks (256 features each)
SLOTS = 8
N_CORES = 8

# Base set: {0,1,2,4} is a perfect difference basis of Z8 (all 7 nonzero
# ordered differences), lifted to Z16.
S_BASE = [0, 1, 2, 4, 8, 9, 10, 12]

# Slot-pair groups, emitted after their gating slot's phase 0. Within a
# group the k-loop interleaves all pairs so same-si runs share LDWEIGHTS.
GROUPS_AFTER_SLOT = {
    0: [[(0, 0)]],
    1: [[(0, 1)]],
    2: [[(0, 2)]],
    3: [[(0, 3)], [(1, 3)]],
    4: [[(4, 1), (4, 2), (4, 4)], [(0, 4)]],
    5: [[(5, 0), (5, 3)], [(4, 5)]],
    6: [[(4, 6), (0, 6)]],
    7: [[(7, 4), (7, 5), (7, 1)]],
}
PAIRS = [p for s in range(SLOTS) for g in GROUPS_AFTER_SLOT.get(s, [])
         for p in g]
assert len(PAIRS) == 17


def slot_blocks(core):
    """Global block index for each slot on a given core."""
    return [(core + S_BASE[k]) % NB for k in range(SLOTS)]


def pair_weight(si, sj):
    """Host-side weight: diagonal loops 1, every cross pair 2 (each
    unordered block pair is computed exactly once globally)."""
    return 1.0 if si == sj else 2.0


def _check_cover():
    """Every unordered cross block-pair hit exactly once; diag once."""
    cross = {}
    diag = {}
    for c in range(N_CORES):
        blocks = slot_blocks(c)
        for (si, sj) in PAIRS:
            a, b = blocks[si], blocks[sj]
            if si == sj:
                diag[a] = diag.get(a, 0) + 1
            else:
                key = (min(a, b), max(a, b))
                cross[key] = cross.get(key, 0) + 1
    assert sorted(diag) == list(range(NB)) and set(diag.values()) == {1}, diag
    assert len(cross) == NB * (NB - 1) // 2 and set(cross.values()) == {1}
_check_cover()


HT = 2            # mi-halves per block: epilogue granularity [128, 2*RPB]
# Per-slot count of subtile squares run on ACT (rest on DVE). Front-loaded:
# ACT is idle before epilogues start, loaded once matmul groups flow. Early
# slots split 2/2 so the two engines square in parallel (lower latency).
SQ_ACT_PER_SLOT = [2, 2, 2, 2, 2, 2, 1, 0]
# Slots whose load is split into 4 subtile DMAs (lower first-byte latency
# at the cost of 3 extra descriptors-gen rounds); later slots load in one.
SPLIT_LOAD_SLOTS = 1


def build_program(psum_bufs=4, sq_act_per_slot=None):
    """Uniform SPMD program. Input: xin [SLOTS, RPB, D] f32 (per-core
    blocks). Output: partials [128, HT*17] f32.

    The reference's max(d2,0) clamp is dropped: it only bites on the
    true-diagonal elements where fp8/bf16 rounding makes c_ii = 1 +- ~1e-2,
    so each of the 8192 diagonal terms is exp(2*delta) ~ 1 +- 2e-2 instead
    of exactly 1; the loss total is ~9.1e6, so the induced error is ~2e-6
    relative - far below the bf16/fp8 noise floor elsewhere.
    """
    nc = bass.Bass()
    xin = nc.declare_dram_parameter("xin", [SLOTS, RPB, D], F32,
                                    isOutput=False)
    pout = nc.declare_dram_parameter(
        "partials", [128, HT * len(PAIRS)], F32, isOutput=True
    )

    mult = mybir.AluOpType.mult
    add = mybir.AluOpType.add
    Exp = mybir.ActivationFunctionType.Exp
    Ln = mybir.ActivationFunctionType.Ln
    Square = mybir.ActivationFunctionType.Square
    # SwInterleave: pairs (2p, 2p+1) stream per column with columns applied
    # in REVERSE order (HW-verified: out[m,n] = sum_j W[:,j,::-1].T X[:,j]).
    # The column reversal only permutes output partitions, which is
    # irrelevant here because the epilogue reduces over partitions.
    DR = mybir.MatmulPerfMode.DoubleRowSwInterleave

    pair_col = {p: i for i, p in enumerate(PAIRS)}
    esc = 2.0 / (FP8_SCALE * FP8_SCALE)
    if sq_act_per_slot is None:
        sq_act_per_slot = SQ_ACT_PER_SLOT

    with TileContext(nc) as tc:
        with (
            tc.tile_pool(name="xnt", bufs=1) as xnt_pool,
            tc.tile_pool(name="stage", bufs=5) as stage_pool,
            tc.tile_pool(name="xq", bufs=6) as xq_pool,
            tc.tile_pool(name="dump", bufs=4) as dump_pool,
            tc.tile_pool(name="edump", bufs=3) as edump_pool,
            tc.tile_pool(name="small", bufs=6) as small_pool,
            tc.tile_pool(name="acc", bufs=1) as acc_pool,
            tc.tile_pool(name="gpsum", bufs=psum_bufs, space="PSUM") as gpsum,
        ):
            partials = acc_pool.tile([128, HT * len(PAIRS)], F32,
                                     tag="partials")
            bias_t = acc_pool.tile([128, 1], F32, tag="biasneg2")
            nc.vector.memset(bias_t, -2.0)
            lnS = acc_pool.tile([128, 1], F32, tag="lnS")
            nc.vector.memset(lnS, float(np.log(FP8_SCALE)))
            # fp8 pair-major transposed blocks: [128, KC, RPB rows, 2 fp8]
            xnt = [
                xnt_pool.tile([128, KC, RPB * 2], FP8, tag=f"xnt{s}",
                              name=f"xnt{s}")
                for s in range(SLOTS)
            ]

            pending_epi = []

            def emit_group(grp):
                for h in range(HT):
                    gs = [
                        gpsum.tile([128, 2 * RPB], F32, tag="g",
                                   name=f"g{si}_{sj}_{h}")
                        for (si, sj) in grp
                    ]
                    for half in range(2):
                        mi = 2 * h + half
                        for k in range(KC):
                            for j, (si, sj) in enumerate(grp):
                                lhsT = xnt[si][:, k,
                                               mi * 256:(mi + 1) * 256] \
                                    .rearrange("p (m j) -> p j m", j=2)
                                rhs = xnt[sj][:, k, :] \
                                    .rearrange("p (n j) -> p j n", j=2)
                                nc.tensor.matmul(
                                    gs[j][:, half * RPB:(half + 1) * RPB],
                                    lhsT, rhs,
                                    start=(k == 0), stop=(k == KC - 1),
                                    perf_mode=DR,
                                )
                    for j, (si, sj) in enumerate(grp):
                        pending_epi.append((gs[j], pair_col[(si, sj)], h))

            def flush_epilogues():
                # Deferred so the exp's PSUM wait doesn't head-of-line
                # block later phase-0 work on the ACT queue.
                for g, pc, h in pending_epi:
                    col = pc * HT + h
                    ed = edump_pool.tile([128, 2 * RPB], BF16, tag="edump")
                    nc.scalar.activation(
                        ed, g, Exp, bias=bias_t, scale=esc,
                        accum_out=partials[:, col:col + 1],
                    )
                pending_epi.clear()

            for s in range(SLOTS):
                # ---- phase 0 for slot s ----
                ssqb = small_pool.tile([128, RT], F32, tag="ssqb")
                # block load with SWDGE f32 -> bf16 cast;
                # xb3[p, r, d] = xin[s, 128r + p, d]
                xb3 = stage_pool.tile([128, RT, D], BF16, tag="xb")
                if s < SPLIT_LOAD_SLOTS:
                    # 4 subtile DMAs: first rows usable ~4x sooner
                    for r in range(RT):
                        nc.gpsimd.dma_start(
                            out=xb3[:, r, :],
                            in_=xin[s, r * 128:(r + 1) * 128, :],
                        )
                else:
                    nc.gpsimd.dma_start(
                        out=xb3,
                        in_=xin[s].rearrange("(r p) d -> p r d", r=RT),
                    )
                n_act = sq_act_per_slot[s]
                for r in range(RT):
                    acc_col = ssqb[:, r:r + 1]
                    xbr = xb3[:, r, :]
                    if (r % 2 == 0) if n_act == 2 else (r < n_act):
                        # ACT square with fused row-sum accumulate
                        sqd = dump_pool.tile([128, D], BF16, tag="sqdump")
                        nc.scalar.activation(sqd, xbr, Square,
                                             accum_out=acc_col)
                    else:
                        # DVE: 2x square, then 4x copy-sum
                        sqd = dump_pool.tile([128, D], BF16, tag="sqdump")
                        nc.vector.tensor_tensor(out=sqd, in0=xbr,
                                                in1=xbr, op=mult)
                        sqd2 = dump_pool.tile([128, D], BF16, tag="sqdump")
                        nc.vector.tensor_scalar(
                            out=sqd2, in0=sqd, scalar1=1.0, scalar2=0.0,
                            op0=mult, op1=add, accum_out=acc_col,
                        )
                # rinv' = 16*rsqrt(ssq) = exp(-0.5*ln(ssq) + ln 16);
                # Ln+Exp share one ACT table set; keep f32 for the
                # tensor_scalar mult operand requirement
                lssq = small_pool.tile([128, RT], F32, tag="lssq")
                nc.scalar.activation(lssq, ssqb, Ln)
                rinvb = small_pool.tile([128, RT], F32, tag="rinvb")
                nc.scalar.activation(rinvb, lssq, Exp, scale=-0.5, bias=lnS)
                for r in range(RT):
                    # normalize-scale fused into the fp8 cast (2x DVE)
                    xq = xq_pool.tile([128, D], FP8, tag="xq")
                    nc.vector.tensor_scalar(
                        out=xq, in0=xb3[:, r, :], scalar1=rinvb[:, r:r + 1],
                        scalar2=None, op0=mult,
                    )
                    # xbar transpose of fp8 PAIRS as 2-byte units:
                    # xnt[s][p, k, rr, j] = q[rr, 256k + 2p + j]
                    nc.sync.dma_start_transpose(
                        out=xnt[s].bitcast(BF16)[:, :,
                                                 r * 128:(r + 1) * 128],
                        in_=xq.bitcast(BF16),
                    )
                # ---- phase 1 groups gated by slot s; previous groups'
                # epilogues flush first (their matmuls have had a slot's
                # time to finish) ----
                flush_epilogues()
                for grp in GROUPS_AFTER_SLOT.get(s, []):
                    emit_group(grp)

            flush_epilogues()
            nc.sync.dma_start(out=pout[:, :], in_=partials)
    _dedup_ldweights(nc)
    _split_multi_waits(nc)
    return nc


_PROGRAM_CACHE = {}


def _get_program():
    if "nc" not in _PROGRAM_CACHE:
        _PROGRAM_CACHE["nc"] = build_program()
    return _PROGRAM_CACHE["nc"]


def shard_inputs(x):
    """x: [8192, 2048] f32 -> per-core input dicts."""
    blocks = x.reshape(NB, RPB, D)
    in_maps = []
    for c in range(N_CORES):
        sel = np.ascontiguousarray(blocks[slot_blocks(c)])
        in_maps.append({"xin": sel})
    return in_maps


def reduce_partials(results, ht=HT):
    """results: list of dicts with 'partials' [128, ht*17] f32 -> scalar."""
    w = np.array([pair_weight(si, sj) for (si, sj) in PAIRS],
                 dtype=np.float64)
    total = 0.0
    for res in results:
        p = res["partials"].astype(np.float64).reshape(128, len(PAIRS), ht)
        total += (p.sum(axis=(0, 2)) * w).sum()
    return total / (N_ROWS * (N_ROWS - 1))


def kernel(class_centroid: np.ndarray) -> np.ndarray:
    x = np.asarray(class_centroid, dtype=np.float32)
    assert x.shape == (N_ROWS, D)
    nc = _get_program()
    in_maps = shard_inputs(x)
    out = run_bass_kernel_spmd(nc, in_maps, list(range(N_CORES)))
    total = reduce_partials(out.results)
    return np.float32(total)
